# revision 11
# baseline (speedup 1.0000x reference)
"""BlurDownsample (depthwise 4x4 FIR + 2x downsample) on 8 TRN2 NeuronCores.

Contract: kernel(x, f) takes the FULL inputs
    x: [16, 128, 256, 256] float32,  f: [4, 4] float32
and returns the FULL output [16, 128, 128, 128] float32, matching
    upfirdn2d(x, f, down=2, padding=(1, 1), flip_filter=False):
    out[n,c,oy,ox] = sum_{dy,dx in 0..3} f[3-dy, 3-dx] * xpad[2oy+dy, 2ox+dx]
with xpad zero-padded by 1 on every spatial edge.

Sharding: pure data-parallel over the batch — core k processes
x[2k:2k+2]; filter-derived constants are replicated.

Per-core strategy (v3 — SDMA-descriptor + HBM-traffic optimized):
  * Host-side, the flipped filter g = flip(f) is factored by SVD into
    R separable terms g = sum_r ah_r (x) bw_r  (R=1 for the
    outer-product filter the model uses).  Only the H-direction runs
    on the Tensor engine; the W-direction is a 4-tap combine on the
    Scalar + Vector engines.  This cuts Tensor-engine streaming 4x
    vs. the banded-matmul-per-filter-column approach.
  * x is converted to bf16 on the host and uploaded as
    [N, C, 128, 512]: HBM read traffic halves (the 2e-2 rel-err gate
    dwarfs bf16's ~2^-9 rounding), every DMA piece is a contiguous
    row pair, and no in-flight cast is needed so loads ride the
    fast HWDGE (sync-engine) path.  SDMA descriptor handling — not
    HBM bandwidth — capped the previous version.
  * The H-FIR+downsample is polyphase banded matmuls in bf16 over
    row-pair partitions: for row parity e, band B_e[p, oh] =
    ah[2p+e-2oh+1] contracts row pairs p, accumulating
    mid[oh, c2, w] in PSUM (2 channels per matmul, rhs free = 512).
    Zero padding in H is implicit in the bands (built host-side).
  * W-combine per channel quad: out[ox] = sum_dx bw[dx]*mid[2ox-1+dx]
    = one Scalar-engine scaled copy (dx=1, full range) plus three
    Vector scalar_tensor_tensor fused multiply-adds (dx=2 full range,
    dx=0/dx=3 edge-clipped), taps as fp32 per-partition SBUF scalars.
  * Stores (fp32) use the scalar-engine HWDGE ring, separate from
    the load ring.
"""

from contextlib import ExitStack

import numpy as np

import concourse.tile as tile
from concourse import bacc, mybir
from concourse.bass_utils import run_bass_kernel_spmd

F32 = mybir.dt.float32
BF16 = mybir.dt.bfloat16

N_CORES = 8
FW = 4  # filter size


def _build_blur_program(nc, N, C, H, W, R):
    OH, OW = H // 2, W // 2
    P = H // 2              # row pairs = SBUF partitions for the contraction
    W2 = 2 * W              # elements per partition row-pair
    CG = min(C, 16)         # channels per load/store group
    QC = 2                  # channels per matmul (PSUM bank: N*4B <= 2KB)
    JJ = min(CG // QC, max(1, 8 // (2 * R)))  # matmul quads per PSUM tile
    assert C % CG == 0 and CG % QC == 0 and P == 128 and W == 256

    x_ap = nc.dram_tensor("x", [N, C, P, W2], BF16, kind="ExternalInput").ap()
    bh_ap = nc.dram_tensor("bh", [R, 2, P, OH], BF16, kind="ExternalInput").ap()
    wt_ap = nc.dram_tensor("wt", [P, 4 * R], F32, kind="ExternalInput").ap()
    out_ap = nc.dram_tensor("out", [N, C, OH, OW], F32, kind="ExternalOutput").ap()

    with tile.TileContext(nc) as tc, ExitStack() as ctx:
        const_pool = ctx.enter_context(tc.tile_pool(name="const", bufs=1))
        x_pool = ctx.enter_context(tc.tile_pool(name="xt", bufs=3))
        acc_pool = ctx.enter_context(tc.tile_pool(name="acc", bufs=2))
        psum_pool = ctx.enter_context(tc.tile_pool(name="mid", bufs=2, space="PSUM"))

        # ---- one-time setup: load bands + taps ----
        bh_sb = const_pool.tile([P, R, 2, OH], BF16, tag="bh")
        for r in range(R):
            for e in range(2):
                nc.sync.dma_start(out=bh_sb[:, r, e, :], in_=bh_ap[r, e])
        wt_sb = const_pool.tile([P, 4 * R], F32, tag="wt")
        nc.sync.dma_start(out=wt_sb[:, :], in_=wt_ap)

        # ---- main loop: groups of CG channels ----
        for n in range(N):
            for c0 in range(0, C, CG):
                xt = x_pool.tile([P, CG, W2], BF16, tag="xt")
                nc.sync.dma_start(  # 1 KiB contiguous pieces (row pairs)
                    out=xt[:, :, :],
                    in_=x_ap[n, c0 : c0 + CG].rearrange("c p w -> p c w"),
                )
                acc = acc_pool.tile([OH, CG // QC, QC, OW], F32, tag="acc")
                for jq in range(CG // (QC * JJ)):
                    js = slice(JJ * jq, JJ * (jq + 1))
                    a_full = acc[:, js, :, :]
                    a0 = acc[:, js, :, 1:OW]
                    a3 = acc[:, js, :, 0 : OW - 1]
                    for r in range(R):
                        mid = psum_pool.tile([OH, JJ, QC, W], F32, tag=f"mid{r}")
                        for e in range(2):
                            for jj in range(JJ):
                                c1 = QC * (JJ * jq + jj)
                                nc.tensor.matmul(
                                    mid[:, jj, :, :],
                                    lhsT=bh_sb[:, r, e, :],
                                    rhs=xt[:, c1 : c1 + QC, e * W : (e + 1) * W],
                                    start=(e == 0),
                                    stop=(e == 1),
                                )
                        # W-combine: out[ox] += sum_dx bw[dx]*mid[2ox-1+dx]
                        # dx=1: iw = 2ox, full range — Scalar engine
                        if r == 0:
                            nc.scalar.mul(
                                a_full,
                                mid[:, :, :, 0:W:2],
                                wt_sb[:, 4 * r + 1 : 4 * r + 2],
                            )
                        else:
                            nc.vector.scalar_tensor_tensor(
                                a_full,
                                mid[:, :, :, 0:W:2],
                                wt_sb[:, 4 * r + 1 : 4 * r + 2],
                                a_full,
                                op0=mybir.AluOpType.mult,
                                op1=mybir.AluOpType.add,
                            )
                        # dx=2: iw = 2ox+1, full range — Vector
                        nc.vector.scalar_tensor_tensor(
                            a_full,
                            mid[:, :, :, 1:W:2],
                            wt_sb[:, 4 * r + 2 : 4 * r + 3],
                            a_full,
                            op0=mybir.AluOpType.mult,
                            op1=mybir.AluOpType.add,
                        )
                        # dx=0: iw = 2ox-1, ox >= 1 — Vector
                        nc.vector.scalar_tensor_tensor(
                            a0,
                            mid[:, :, :, 1 : W - 2 : 2],
                            wt_sb[:, 4 * r : 4 * r + 1],
                            a0,
                            op0=mybir.AluOpType.mult,
                            op1=mybir.AluOpType.add,
                        )
                        # dx=3: iw = 2ox+2, ox <= OW-2 — Vector
                        nc.vector.scalar_tensor_tensor(
                            a3,
                            mid[:, :, :, 2 : W - 1 : 2],
                            wt_sb[:, 4 * r + 3 : 4 * r + 4],
                            a3,
                            op0=mybir.AluOpType.mult,
                            op1=mybir.AluOpType.add,
                        )
                nc.scalar.dma_start(
                    out=out_ap[n, c0 : c0 + CG].rearrange("c oh ow -> oh c ow"),
                    in_=acc[:, :, :, :].rearrange("p a b w -> p (a b) w"),
                )
    return nc


def _factor_filter(f):
    """Factor the flipped filter into R separable (ah, bw) term pairs."""
    g = np.flip(np.asarray(f, dtype=np.float64))
    U, s, Vt = np.linalg.svd(g)
    if s[0] <= 0.0:
        return 0, None, None
    R = int(np.sum(s > s[0] * 1e-4))
    ah = (U[:, :R] * np.sqrt(s[:R])).astype(np.float32)        # [4, R]
    bw = (Vt[:R, :].T * np.sqrt(s[:R])).astype(np.float32)     # [4, R]
    return R, ah, bw


def _build_inputs(ah, bw, P, OH, R):
    bh = np.zeros((R, 2, P, OH), dtype=np.float32)
    for r in range(R):
        for e in range(2):
            for d in range(-2, 3):  # oh = p - d; band is narrow
                dy = 2 * d + e + 1
                if 0 <= dy < FW:
                    idx = np.arange(max(0, d), min(P, OH + d))
                    bh[r, e, idx, idx - d] = ah[dy, r]
    wt = np.tile(bw.T.reshape(1, 4 * R), (P, 1)).astype(np.float32)
    return bh, wt


_PROGRAM_CACHE = {}


def _get_program(shape, R):
    key = (shape, R)
    if key not in _PROGRAM_CACHE:
        N, C, H, W = shape
        nb = N // N_CORES
        nc = bacc.Bacc(
            "TRN2", target_bir_lowering=False, debug=False, num_devices=N_CORES
        )
        _build_blur_program(nc, nb, C, H, W, R)
        nc.compile()
        _PROGRAM_CACHE[key] = nc
    return _PROGRAM_CACHE[key]


def _run(x, f, trace=False, tmpdir=None):
    x = np.ascontiguousarray(x, dtype=np.float32)
    f = np.ascontiguousarray(f, dtype=np.float32)
    N, C, H, W = x.shape
    OH, OW = H // 2, W // 2
    assert N % N_CORES == 0, f"batch {N} not divisible by {N_CORES} cores"
    nb = N // N_CORES

    R, ah, bw = _factor_filter(f)
    if R == 0:
        return np.zeros((N, C, OH, OW), dtype=np.float32), None
    bh, wt = _build_inputs(ah, bw, H // 2, OH, R)

    nc = _get_program((N, C, H, W), R)
    np_bf16 = mybir.dt.np(BF16)
    xv = np.ascontiguousarray(
        x.reshape(N, C, H // 2, 2 * W).astype(np_bf16)
    )
    bhv = bh.astype(np_bf16)
    in_maps = [
        {"x": xv[k * nb : (k + 1) * nb], "bh": bhv, "wt": wt}
        for k in range(N_CORES)
    ]
    res = run_bass_kernel_spmd(
        nc, in_maps, core_ids=list(range(N_CORES)), trace=trace, tmpdir=tmpdir
    )
    out = np.concatenate(
        [res.results[k]["out"] for k in range(N_CORES)], axis=0
    )
    return out, res


def kernel(x, f):
    out, _ = _run(x, f)
    return out
